# revision 9
# baseline (speedup 1.0000x reference)
"""Sparse-attention Trainium2 kernel, 8-way data-parallel over batch.

Reference computation (per batch):
  qkv = x @ qkv_w.T + qkv_b              -> split q,k,v [H=12, N=388, D=64]
  template queries (tokens 0:128) attend to template keys (0:128)
  search queries (tokens 128:388) attend to all 388 keys
  out = concat @ proj_w.T + proj_b

Kernel strategy per core (B_local=8 batches, all compute on device):
  - x^T (feature-major) built via PE transposes.
  - q^T,k^T = W^T-stationary matmuls (feature-major out, per-partition bias on ACT).
  - v = x^T-stationary matmuls (token-major out), stored per-head with a ones
    column appended so the attention-value matmul also produces softmax sums.
  - scores computed TRANSPOSED: S^T[k,q] = (k^T)^T-slices as lhsT, q^T as rhs.
    exp on ACT (scale=1/8 folded in), probs in bf16.
  - AV: out^T[d,q] accumulated over k-chunks; row 64 = softmax denominators.
  - normalize: reciprocal on DVE, partition_broadcast on GpSimd, multiply on DVE.
  - attention output is feature-major -> proj matmul directly, bias on DVE,
    DMA out token-major.
"""

import numpy as np

B, N, C = 64, 388, 768
H, D = 12, 64
LT = 128          # template tokens (= first token chunk, exactly)
LS = N - LT       # 260 search tokens
NCORES = 8
BL = B // NCORES  # 8 batches per core
O3 = 3 * C        # 2304
SCALE = 0.125

_NC_CACHE = {}


def _build_nc(dump=False):
    from contextlib import ExitStack

    import concourse.tile as tile
    from concourse import bacc, mybir
    from concourse.masks import make_identity

    f32 = mybir.dt.float32
    f32r = mybir.dt.float32r
    bf16 = mybir.dt.bfloat16
    Identity = mybir.ActivationFunctionType.Identity
    Exp = mybir.ActivationFunctionType.Exp
    mult = mybir.AluOpType.mult
    add = mybir.AluOpType.add

    nc = bacc.Bacc("TRN2", target_bir_lowering=False)

    x_ext = nc.dram_tensor("x", [BL, N, C], f32r, kind="ExternalInput")
    qkvw_ext = nc.dram_tensor("qkv_w", [O3, C], f32r, kind="ExternalInput")
    qkvb_ext = nc.dram_tensor("qkv_b", [O3], f32r, kind="ExternalInput")
    projw_ext = nc.dram_tensor("proj_w", [C, C], f32r, kind="ExternalInput")
    projb_ext = nc.dram_tensor("proj_b", [C], f32, kind="ExternalInput")
    out_ext = nc.dram_tensor("out", [BL, N, C], f32, kind="ExternalOutput")
    if dump:
        dxt_ext = nc.dram_tensor("d_xt", [128, 6, N], f32, kind="ExternalOutput")
        dq_ext = nc.dram_tensor("d_q", [128, 6, N], bf16, kind="ExternalOutput")
        dk_ext = nc.dram_tensor("d_k", [128, 6, N], bf16, kind="ExternalOutput")
        dv_ext = nc.dram_tensor("d_v", [128, 4, H, 65], bf16, kind="ExternalOutput")
        da_ext = nc.dram_tensor("d_at", [128, 6, N], f32, kind="ExternalOutput")
        dpav_ext = nc.dram_tensor("d_pav", [65, N], f32, kind="ExternalOutput")
        drb_ext = nc.dram_tensor("d_rb", [64, N], f32, kind="ExternalOutput")

    # token chunking of the 388 tokens: 128,128,128,4
    TCH = [(0, 128), (128, 128), (256, 128), (384, 4)]

    with tile.TileContext(nc) as tc, ExitStack() as ctx:
        const = ctx.enter_context(tc.tile_pool(name="const", bufs=1))
        stage = ctx.enter_context(tc.tile_pool(name="stage", bufs=2))
        psmall = ctx.enter_context(tc.tile_pool(name="psmall", bufs=4, space="PSUM"))
        pbig = ctx.enter_context(tc.tile_pool(name="pbig", bufs=2, space="PSUM"))

        ident_f = const.tile([128, 128], f32)
        make_identity(nc, ident_f)
        ident = const.tile([128, 128], f32r)
        nc.vector.tensor_copy(out=ident[:], in_=ident_f[:])
        idr = ident[:]

        # ---- weights: transpose qkv_w [2304,768] -> wT [c-part, ct, o] ----
        wT = const.tile([128, 6, O3], f32r)
        for j in range(18):
            wst = stage.tile([128, C], f32r, tag="wst")
            nc.sync.dma_start(out=wst[:], in_=qkvw_ext[j * 128:(j + 1) * 128, :])
            for ct in range(6):
                pt = psmall.tile([128, 128], f32r, tag="sp")
                nc.tensor.transpose(
                    pt[:], wst[:, ct * 128:(ct + 1) * 128], idr
                )
                nc.scalar.copy(out=wT[:, ct, j * 128:(j + 1) * 128],
                               in_=pt[:])

        projT = const.tile([128, 6, C], f32r)
        for j in range(6):
            pst_w = stage.tile([128, C], f32r, tag="wst")
            nc.sync.dma_start(out=pst_w[:], in_=projw_ext[j * 128:(j + 1) * 128, :])
            for ct in range(6):
                pt = psmall.tile([128, 128], f32r, tag="sp")
                nc.tensor.transpose(
                    pt[:], pst_w[:, ct * 128:(ct + 1) * 128], idr
                )
                nc.scalar.copy(out=projT[:, ct, j * 128:(j + 1) * 128],
                               in_=pt[:])

        # ---- biases ----
        # q/k bias as per-partition columns [128, 12] (o-tiles 0..11)
        qb_st = stage.tile([12, 128], f32r, tag="bst")
        nc.sync.dma_start(out=qb_st[:], in_=qkvb_ext[0:1536].rearrange("(j p) -> j p", p=128))
        pbt = psmall.tile([128, 12], f32r, tag="sp")
        nc.tensor.transpose(pbt[:], qb_st[:], idr[0:12, 0:12])
        qkb_sb = const.tile([128, 12], f32)
        nc.scalar.copy(out=qkb_sb[:], in_=pbt[:].bitcast(f32))
        # v bias broadcast along partitions [128, 768]
        vb_bc = const.tile([128, C], f32r)
        nc.sync.dma_start(out=vb_bc[:], in_=qkvb_ext[1536:2304].unsqueeze(0).to_broadcast([128, C]))
        # proj bias broadcast
        pb_bc = const.tile([128, C], f32)
        nc.sync.dma_start(out=pb_bc[:], in_=projb_ext[:].unsqueeze(0).to_broadcast([128, C]))

        # ---- per-batch pools ----
        xpool = ctx.enter_context(tc.tile_pool(name="xp", bufs=1))
        xtpool = ctx.enter_context(tc.tile_pool(name="xtp", bufs=2))
        qkpool = ctx.enter_context(tc.tile_pool(name="qkp", bufs=2))
        vpool = ctx.enter_context(tc.tile_pool(name="vp", bufs=2))
        apool = ctx.enter_context(tc.tile_pool(name="ap", bufs=2))
        ppool = ctx.enter_context(tc.tile_pool(name="pp", bufs=2))
        spool = ctx.enter_context(tc.tile_pool(name="ssp", bufs=2))
        opool = ctx.enter_context(tc.tile_pool(name="op", bufs=2))

        for b in range(BL):
            # load x_b and transpose to feature-major xT [128c, ct, t]
            xb = xpool.tile([128, 4, C], f32r, tag="xb")
            nc.sync.dma_start(
                out=xb[:, 0:3, :],
                in_=x_ext[b, 0:384, :].rearrange("(t p) c -> p t c", p=128),
            )
            nc.sync.dma_start(out=xb[0:4, 3, :], in_=x_ext[b, 384:388, :])

            xTb = xtpool.tile([128, 6, N], f32r, tag="xt")
            for ti, (t0, tp) in enumerate(TCH):
                for ct in range(6):
                    pt = psmall.tile([128, 128], f32r, tag="sp")
                    nc.tensor.transpose(
                        pt[:, 0:tp],
                        xb[0:tp, ti, ct * 128:(ct + 1) * 128],
                        idr[0:tp, 0:tp],
                    )
                    nc.scalar.copy(out=xTb[:, ct, t0:t0 + tp],
                                   in_=pt[:, 0:tp])

            # ---- q^T, k^T (feature-major, bf16, bias added) ----
            qTb = qkpool.tile([128, 6, N], bf16, tag="q")
            kTb = qkpool.tile([128, 6, N], bf16, tag="k")
            for j in range(12):
                ps = psmall.tile([128, N], f32, tag="sp")
                for ct in range(6):
                    nc.tensor.matmul(
                        ps[:],
                        lhsT=wT[:, ct, j * 128:(j + 1) * 128],
                        rhs=xTb[:, ct, :],
                        start=(ct == 0), stop=(ct == 5),
                    )
                dst = qTb[:, j, :] if j < 6 else kTb[:, j - 6, :]
                nc.scalar.activation(out=dst, in_=ps[:], func=Identity,
                                     bias=qkb_sb[:, j:j + 1], scale=1.0)

            # ---- v (token-major, per-head with ones column) ----
            vb = vpool.tile([128, 4, H, 65], bf16, tag="v")
            nc.vector.memset(vb[:, :, :, 64:65], 1.0)
            for ti, (t0, tp) in enumerate(TCH):
                pv = pbig.tile([128, C], f32, tag="bp")
                for o0, on in ((0, 512), (512, 256)):   # PSUM-bank-aligned splits
                    for ct in range(6):
                        nc.tensor.matmul(
                            pv[0:tp, o0:o0 + on],
                            lhsT=xTb[:, ct, t0:t0 + tp],
                            rhs=wT[:, ct, 1536 + o0:1536 + o0 + on],
                            start=(ct == 0), stop=(ct == 5),
                        )
                nc.vector.tensor_tensor(
                    out=vb[0:tp, ti, :, 0:64],
                    in0=pv[0:tp, :].rearrange("p (h d) -> p h d", h=H),
                    in1=vb_bc[0:tp, :].bitcast(f32).rearrange("p (h d) -> p h d", h=H),
                    op=add,
                )

            # ---- attention per head ----
            xattnT = apool.tile([128, 6, N], f32r, tag="xat")
            for h in range(H):
                cth, r0 = h // 2, (h % 2) * 64
                qh = qTb[r0:r0 + 64, cth, :]   # [64, 388] bf16
                kh = kTb[r0:r0 + 64, cth, :]

                pav = psmall.tile([65, N], f32, tag="sp")

                # template: S^T = k_mt^T-major scores [128k, 128q]
                pst = psmall.tile([128, 128], f32, tag="sp")
                nc.tensor.matmul(pst[:], lhsT=kh[:, 0:LT], rhs=qh[:, 0:LT],
                                 start=True, stop=True)
                prt = ppool.tile([128, 128], bf16, tag="prt")
                nc.scalar.activation(out=prt[:], in_=pst[:], func=Exp, scale=SCALE)
                nc.tensor.matmul(pav[:, 0:LT], lhsT=vb[:, 0, h, :], rhs=prt[:],
                                 start=True, stop=True)

                # search: all 388 keys in 4 chunks, queries 128:388
                for kc, (t0, tp) in enumerate(TCH):
                    pss = psmall.tile([128, LS], f32, tag="sp")
                    nc.tensor.matmul(pss[0:tp, :], lhsT=kh[:, t0:t0 + tp],
                                     rhs=qh[:, LT:N], start=True, stop=True)
                    prs = ppool.tile([128, LS], bf16, tag="prs")
                    nc.scalar.activation(out=prs[0:tp, :], in_=pss[0:tp, :],
                                         func=Exp, scale=SCALE)
                    nc.tensor.matmul(pav[:, LT:N], lhsT=vb[0:tp, kc, h, :],
                                     rhs=prs[0:tp, :],
                                     start=(kc == 0), stop=(kc == 3))

                if dump and b == 0 and h == 0:
                    pavf = spool.tile([65, N], f32, tag="pavf")
                    nc.vector.tensor_copy(out=pavf[:], in_=pav[:])
                    nc.sync.dma_start(out=dpav_ext[:], in_=pavf[:])
                rinv = spool.tile([1, N], f32, tag="ri")
                nc.vector.reciprocal(out=rinv[:], in_=pav[64:65, :])
                rb = spool.tile([64, N], f32, tag="rb")
                nc.gpsimd.partition_broadcast(rb[:], rinv[:])
                if dump and b == 0 and h == 0:
                    nc.sync.dma_start(out=drb_ext[:], in_=rb[:])
                nc.vector.tensor_tensor(out=xattnT[r0:r0 + 64, cth, :],
                                        in0=pav[0:64, :], in1=rb[:], op=mult)

            if dump and b == 0:
                nc.sync.dma_start(out=dxt_ext[:], in_=xTb[:].bitcast(f32))
                nc.sync.dma_start(out=dq_ext[:], in_=qTb[:])
                nc.sync.dma_start(out=dk_ext[:], in_=kTb[:])
                nc.sync.dma_start(out=dv_ext[:], in_=vb[:])
                nc.sync.dma_start(out=da_ext[:], in_=xattnT[:].bitcast(f32))

            # ---- proj + bias + store ----
            for ti, (t0, tp) in enumerate(TCH):
                pp = pbig.tile([128, C], f32, tag="bp")
                for o0, on in ((0, 512), (512, 256)):   # PSUM-bank-aligned splits
                    for ct in range(6):
                        nc.tensor.matmul(
                            pp[0:tp, o0:o0 + on],
                            lhsT=xattnT[:, ct, t0:t0 + tp],
                            rhs=projT[:, ct, o0:o0 + on],
                            start=(ct == 0), stop=(ct == 5),
                        )
                osb = opool.tile([128, C], f32, tag="ob")
                nc.vector.tensor_tensor(out=osb[0:tp, :], in0=pp[0:tp, :],
                                        in1=pb_bc[0:tp, :], op=add)
                nc.sync.dma_start(out=out_ext[b, t0:t0 + tp, :], in_=osb[0:tp, :])

    nc.compile()
    return nc


def _get_nc():
    if "nc" not in _NC_CACHE:
        _NC_CACHE["nc"] = _build_nc()
    return _NC_CACHE["nc"]


def kernel(x, qkv_w, qkv_b, proj_w, proj_b, t_h=8, t_w=8, s_h=16, s_w=16):
    from concourse.bass_utils import run_bass_kernel_spmd

    x = np.ascontiguousarray(np.asarray(x, dtype=np.float32))
    qkv_w = np.ascontiguousarray(np.asarray(qkv_w, dtype=np.float32))
    qkv_b = np.ascontiguousarray(np.asarray(qkv_b, dtype=np.float32))
    proj_w = np.ascontiguousarray(np.asarray(proj_w, dtype=np.float32))
    proj_b = np.ascontiguousarray(np.asarray(proj_b, dtype=np.float32))

    nc = _get_nc()
    in_maps = [
        {
            "x": x[i * BL:(i + 1) * BL],
            "qkv_w": qkv_w,
            "qkv_b": qkv_b,
            "proj_w": proj_w,
            "proj_b": proj_b,
        }
        for i in range(NCORES)
    ]
    res = run_bass_kernel_spmd(nc, in_maps, core_ids=list(range(NCORES)))
    out = np.concatenate([res.results[i]["out"] for i in range(NCORES)], axis=0)
    return out.astype(np.float32)


# revision 10
# speedup vs baseline: 7705.9420x; 7705.9420x over previous
"""Sparse-attention Trainium2 kernel, 8-way data-parallel over batch.

Reference computation (per batch):
  qkv = x @ qkv_w.T + qkv_b              -> split q,k,v [H=12, N=388, D=64]
  template queries (tokens 0:128) attend to template keys (0:128)
  search queries (tokens 128:388) attend to all 388 keys
  out = concat @ proj_w.T + proj_b

Kernel strategy per core (B_local=8 batches, all compute on device):
  - x^T (feature-major) built via PE transposes.
  - q^T,k^T = W^T-stationary matmuls (feature-major out, per-partition bias on ACT).
  - v = x^T-stationary matmuls (token-major out), stored per-head with a ones
    column appended so the attention-value matmul also produces softmax sums.
  - scores computed TRANSPOSED: S^T[k,q] = (k^T)^T-slices as lhsT, q^T as rhs.
    exp on ACT (scale=1/8 folded in), probs in bf16.
  - AV: out^T[d,q] accumulated over k-chunks; row 64 = softmax denominators.
  - normalize: reciprocal on DVE, partition_broadcast on GpSimd, multiply on DVE.
  - attention output is feature-major -> proj matmul directly, bias on DVE,
    DMA out token-major.
"""

import numpy as np

B, N, C = 64, 388, 768
H, D = 12, 64
LT = 128          # template tokens (= first token chunk, exactly)
LS = N - LT       # 260 search tokens
NCORES = 8
BL = B // NCORES  # 8 batches per core
O3 = 3 * C        # 2304
SCALE = 0.125

_NC_CACHE = {}


def _build_nc(dump=False, reps=1):
    from contextlib import ExitStack

    import concourse.tile as tile
    from concourse import bacc, mybir
    from concourse.masks import make_identity

    f32 = mybir.dt.float32
    f32r = mybir.dt.float32r
    bf16 = mybir.dt.bfloat16
    Identity = mybir.ActivationFunctionType.Identity
    Exp = mybir.ActivationFunctionType.Exp
    mult = mybir.AluOpType.mult
    add = mybir.AluOpType.add

    nc = bacc.Bacc("TRN2", target_bir_lowering=False)

    x_ext = nc.dram_tensor("x", [BL, N, C], f32r, kind="ExternalInput")
    qkvw_ext = nc.dram_tensor("qkv_w", [O3, C], f32r, kind="ExternalInput")
    qkvb_ext = nc.dram_tensor("qkv_b", [O3], f32r, kind="ExternalInput")
    projw_ext = nc.dram_tensor("proj_w", [C, C], f32r, kind="ExternalInput")
    projb_ext = nc.dram_tensor("proj_b", [C], f32, kind="ExternalInput")
    out_ext = nc.dram_tensor("out", [BL, N, C], f32, kind="ExternalOutput")
    if dump:
        dxt_ext = nc.dram_tensor("d_xt", [128, 6, N], f32, kind="ExternalOutput")
        dq_ext = nc.dram_tensor("d_q", [128, 6, N], bf16, kind="ExternalOutput")
        dk_ext = nc.dram_tensor("d_k", [128, 6, N], bf16, kind="ExternalOutput")
        dv_ext = nc.dram_tensor("d_v", [128, 4, H, 65], bf16, kind="ExternalOutput")
        da_ext = nc.dram_tensor("d_at", [128, 6, N], f32, kind="ExternalOutput")
        dpav_ext = nc.dram_tensor("d_pav", [65, N], f32, kind="ExternalOutput")
        drb_ext = nc.dram_tensor("d_rb", [64, N], f32, kind="ExternalOutput")

    # token chunking of the 388 tokens: 128,128,128,4
    TCH = [(0, 128), (128, 128), (256, 128), (384, 4)]

    with tile.TileContext(nc) as tc, ExitStack() as ctx:
        const = ctx.enter_context(tc.tile_pool(name="const", bufs=1))
        stage = ctx.enter_context(tc.tile_pool(name="stage", bufs=2))
        psmall = ctx.enter_context(tc.tile_pool(name="psmall", bufs=4, space="PSUM"))
        pbig = ctx.enter_context(tc.tile_pool(name="pbig", bufs=2, space="PSUM"))

        ident_f = const.tile([128, 128], f32)
        make_identity(nc, ident_f)
        ident = const.tile([128, 128], f32r)
        nc.vector.tensor_copy(out=ident[:], in_=ident_f[:])
        idr = ident[:]

        # ---- weights: transpose qkv_w [2304,768] -> wT [c-part, ct, o] ----
        wT = const.tile([128, 6, O3], f32r)
        for j in range(18):
            wst = stage.tile([128, C], f32r, tag="wst")
            nc.sync.dma_start(out=wst[:], in_=qkvw_ext[j * 128:(j + 1) * 128, :])
            for ct in range(6):
                pt = psmall.tile([128, 128], f32r, tag="sp")
                nc.tensor.transpose(
                    pt[:], wst[:, ct * 128:(ct + 1) * 128], idr
                )
                nc.scalar.copy(out=wT[:, ct, j * 128:(j + 1) * 128],
                               in_=pt[:])

        projT = const.tile([128, 6, C], f32r)
        for j in range(6):
            pst_w = stage.tile([128, C], f32r, tag="wst")
            nc.sync.dma_start(out=pst_w[:], in_=projw_ext[j * 128:(j + 1) * 128, :])
            for ct in range(6):
                pt = psmall.tile([128, 128], f32r, tag="sp")
                nc.tensor.transpose(
                    pt[:], pst_w[:, ct * 128:(ct + 1) * 128], idr
                )
                nc.scalar.copy(out=projT[:, ct, j * 128:(j + 1) * 128],
                               in_=pt[:])

        # ---- biases ----
        # q/k bias as per-partition columns [128, 12] (o-tiles 0..11)
        qb_st = stage.tile([12, 128], f32r, tag="bst")
        nc.sync.dma_start(out=qb_st[:], in_=qkvb_ext[0:1536].rearrange("(j p) -> j p", p=128))
        pbt = psmall.tile([128, 12], f32r, tag="sp")
        nc.tensor.transpose(pbt[:], qb_st[:], idr[0:12, 0:12])
        qkb_sb = const.tile([128, 12], f32)
        nc.scalar.copy(out=qkb_sb[:], in_=pbt[:].bitcast(f32))
        # v bias broadcast along partitions [128, 768]
        vb_bc = const.tile([128, C], f32r)
        nc.sync.dma_start(out=vb_bc[:], in_=qkvb_ext[1536:2304].unsqueeze(0).to_broadcast([128, C]))
        # proj bias broadcast
        pb_bc = const.tile([128, C], f32)
        nc.sync.dma_start(out=pb_bc[:], in_=projb_ext[:].unsqueeze(0).to_broadcast([128, C]))

        # ---- per-batch pools ----
        xpool = ctx.enter_context(tc.tile_pool(name="xp", bufs=1))
        xtpool = ctx.enter_context(tc.tile_pool(name="xtp", bufs=2))
        qkpool = ctx.enter_context(tc.tile_pool(name="qkp", bufs=2))
        vpool = ctx.enter_context(tc.tile_pool(name="vp", bufs=2))
        apool = ctx.enter_context(tc.tile_pool(name="ap", bufs=2))
        ppool = ctx.enter_context(tc.tile_pool(name="pp", bufs=2))
        spool = ctx.enter_context(tc.tile_pool(name="ssp", bufs=2))
        opool = ctx.enter_context(tc.tile_pool(name="op", bufs=2))

        for b in [bb for _ in range(reps) for bb in range(BL)]:
            # load x_b and transpose to feature-major xT [128c, ct, t]
            xb = xpool.tile([128, 4, C], f32r, tag="xb")
            nc.sync.dma_start(
                out=xb[:, 0:3, :],
                in_=x_ext[b, 0:384, :].rearrange("(t p) c -> p t c", p=128),
            )
            nc.sync.dma_start(out=xb[0:4, 3, :], in_=x_ext[b, 384:388, :])

            xTb = xtpool.tile([128, 6, N], f32r, tag="xt")
            for ti, (t0, tp) in enumerate(TCH):
                for ct in range(6):
                    pt = psmall.tile([128, 128], f32r, tag="sp")
                    nc.tensor.transpose(
                        pt[:, 0:tp],
                        xb[0:tp, ti, ct * 128:(ct + 1) * 128],
                        idr[0:tp, 0:tp],
                    )
                    nc.scalar.copy(out=xTb[:, ct, t0:t0 + tp],
                                   in_=pt[:, 0:tp])

            # ---- q^T, k^T (feature-major, bf16, bias added) ----
            qTb = qkpool.tile([128, 6, N], bf16, tag="q")
            kTb = qkpool.tile([128, 6, N], bf16, tag="k")
            for j in range(12):
                ps = psmall.tile([128, N], f32, tag="sp")
                for ct in range(6):
                    nc.tensor.matmul(
                        ps[:],
                        lhsT=wT[:, ct, j * 128:(j + 1) * 128],
                        rhs=xTb[:, ct, :],
                        start=(ct == 0), stop=(ct == 5),
                    )
                dst = qTb[:, j, :] if j < 6 else kTb[:, j - 6, :]
                nc.scalar.activation(out=dst, in_=ps[:], func=Identity,
                                     bias=qkb_sb[:, j:j + 1], scale=1.0)

            # ---- v (token-major, per-head with ones column) ----
            vb = vpool.tile([128, 4, H, 65], bf16, tag="v")
            nc.vector.memset(vb[:, :, :, 64:65], 1.0)
            for ti, (t0, tp) in enumerate(TCH):
                pv = pbig.tile([128, C], f32, tag="bp")
                for o0, on in ((0, 512), (512, 256)):   # PSUM-bank-aligned splits
                    for ct in range(6):
                        nc.tensor.matmul(
                            pv[0:tp, o0:o0 + on],
                            lhsT=xTb[:, ct, t0:t0 + tp],
                            rhs=wT[:, ct, 1536 + o0:1536 + o0 + on],
                            start=(ct == 0), stop=(ct == 5),
                        )
                nc.vector.tensor_tensor(
                    out=vb[0:tp, ti, :, 0:64],
                    in0=pv[0:tp, :].rearrange("p (h d) -> p h d", h=H),
                    in1=vb_bc[0:tp, :].bitcast(f32).rearrange("p (h d) -> p h d", h=H),
                    op=add,
                )

            # ---- attention per head ----
            xattnT = apool.tile([128, 6, N], f32r, tag="xat")
            for h in range(H):
                cth, r0 = h // 2, (h % 2) * 64
                qh = qTb[r0:r0 + 64, cth, :]   # [64, 388] bf16
                kh = kTb[r0:r0 + 64, cth, :]

                pav = psmall.tile([65, N], f32, tag="sp")

                # template: S^T = k_mt^T-major scores [128k, 128q]
                pst = psmall.tile([128, 128], f32, tag="sp")
                nc.tensor.matmul(pst[:], lhsT=kh[:, 0:LT], rhs=qh[:, 0:LT],
                                 start=True, stop=True)
                prt = ppool.tile([128, 128], bf16, tag="prt")
                nc.scalar.activation(out=prt[:], in_=pst[:], func=Exp, scale=SCALE)
                nc.tensor.matmul(pav[:, 0:LT], lhsT=vb[:, 0, h, :], rhs=prt[:],
                                 start=True, stop=True)

                # search: all 388 keys in 4 chunks, queries 128:388
                for kc, (t0, tp) in enumerate(TCH):
                    pss = psmall.tile([128, LS], f32, tag="sp")
                    nc.tensor.matmul(pss[0:tp, :], lhsT=kh[:, t0:t0 + tp],
                                     rhs=qh[:, LT:N], start=True, stop=True)
                    prs = ppool.tile([128, LS], bf16, tag="prs")
                    nc.scalar.activation(out=prs[0:tp, :], in_=pss[0:tp, :],
                                         func=Exp, scale=SCALE)
                    nc.tensor.matmul(pav[:, LT:N], lhsT=vb[0:tp, kc, h, :],
                                     rhs=prs[0:tp, :],
                                     start=(kc == 0), stop=(kc == 3))

                if dump and b == 0 and h == 0:
                    pavf = spool.tile([65, N], f32, tag="pavf")
                    nc.vector.tensor_copy(out=pavf[:], in_=pav[:])
                    nc.sync.dma_start(out=dpav_ext[:], in_=pavf[:])
                rinv = spool.tile([1, N], f32, tag="ri")
                nc.vector.reciprocal(out=rinv[:], in_=pav[64:65, :])
                rb = spool.tile([64, N], f32, tag="rb")
                nc.gpsimd.partition_broadcast(rb[:], rinv[:])
                if dump and b == 0 and h == 0:
                    nc.sync.dma_start(out=drb_ext[:], in_=rb[:])
                nc.vector.tensor_tensor(out=xattnT[r0:r0 + 64, cth, :],
                                        in0=pav[0:64, :], in1=rb[:], op=mult)

            if dump and b == 0:
                nc.sync.dma_start(out=dxt_ext[:], in_=xTb[:].bitcast(f32))
                nc.sync.dma_start(out=dq_ext[:], in_=qTb[:])
                nc.sync.dma_start(out=dk_ext[:], in_=kTb[:])
                nc.sync.dma_start(out=dv_ext[:], in_=vb[:])
                nc.sync.dma_start(out=da_ext[:], in_=xattnT[:].bitcast(f32))

            # ---- proj + bias + store ----
            for ti, (t0, tp) in enumerate(TCH):
                pp = pbig.tile([128, C], f32, tag="bp")
                for o0, on in ((0, 512), (512, 256)):   # PSUM-bank-aligned splits
                    for ct in range(6):
                        nc.tensor.matmul(
                            pp[0:tp, o0:o0 + on],
                            lhsT=xattnT[:, ct, t0:t0 + tp],
                            rhs=projT[:, ct, o0:o0 + on],
                            start=(ct == 0), stop=(ct == 5),
                        )
                osb = opool.tile([128, C], f32, tag="ob")
                nc.vector.tensor_tensor(out=osb[0:tp, :], in0=pp[0:tp, :],
                                        in1=pb_bc[0:tp, :], op=add)
                nc.sync.dma_start(out=out_ext[b, t0:t0 + tp, :], in_=osb[0:tp, :])

    nc.compile()
    return nc


def _get_nc():
    if "nc" not in _NC_CACHE:
        _NC_CACHE["nc"] = _build_nc()
    return _NC_CACHE["nc"]


def kernel(x, qkv_w, qkv_b, proj_w, proj_b, t_h=8, t_w=8, s_h=16, s_w=16):
    from concourse.bass_utils import run_bass_kernel_spmd

    x = np.ascontiguousarray(np.asarray(x, dtype=np.float32))
    qkv_w = np.ascontiguousarray(np.asarray(qkv_w, dtype=np.float32))
    qkv_b = np.ascontiguousarray(np.asarray(qkv_b, dtype=np.float32))
    proj_w = np.ascontiguousarray(np.asarray(proj_w, dtype=np.float32))
    proj_b = np.ascontiguousarray(np.asarray(proj_b, dtype=np.float32))

    nc = _get_nc()
    in_maps = [
        {
            "x": x[i * BL:(i + 1) * BL],
            "qkv_w": qkv_w,
            "qkv_b": qkv_b,
            "proj_w": proj_w,
            "proj_b": proj_b,
        }
        for i in range(NCORES)
    ]
    res = run_bass_kernel_spmd(nc, in_maps, core_ids=list(range(NCORES)))
    out = np.concatenate([res.results[i]["out"] for i in range(NCORES)], axis=0)
    return out.astype(np.float32)


# revision 17
# speedup vs baseline: 10660.6688x; 1.3834x over previous
"""Sparse-attention Trainium2 kernel, 8-way data-parallel over batch.

Reference computation (per batch):
  qkv = x @ qkv_w.T + qkv_b              -> split q,k,v [H=12, N=388, D=64]
  template queries (tokens 0:128) attend to template keys (0:128)
  search queries (tokens 128:388) attend to all 388 keys
  out = concat @ proj_w.T + proj_b

Kernel strategy per core (B_local=8 batches, all compute on device, bf16
matmuls with fp32 PSUM accumulation):
  - x cast to bf16, transposed feature-major via DMA-xbar transposes.
  - q^T,k^T = W^T-stationary matmuls (feature-major out, per-partition bias
    added in fp32 on ACT during the PSUM->SBUF copy).
  - v = x^T-stationary matmuls (token-major out), stored per-head with a ones
    column appended so the attention-value matmul also produces softmax sums.
  - scores computed TRANSPOSED: S^T[k,q] = k^T-slices as lhsT, q^T as rhs.
    exp on ACT (scale=1/8 folded in), probs in bf16.
  - AV: out^T[d,q] accumulated over k-chunks; row 64 = softmax denominators.
  - normalize: reciprocal on DVE, partition_broadcast on GpSimd, multiply on
    DVE (writes bf16 feature-major attention output).
  - proj matmul reads attention output directly (no transposes), bias on DVE,
    DMA out token-major fp32.
"""

import numpy as np

B, N, C = 64, 388, 768
H, D = 12, 64
LT = 128          # template tokens (= first token chunk, exactly)
LS = N - LT       # 260 search tokens
NCORES = 8
BL = B // NCORES  # 8 batches per core
O3 = 3 * C        # 2304
SCALE = 0.125
NPAD = 416        # 388 tokens padded to 32-multiple for DMA-transpose tiles

_NC_CACHE = {}


def _build_nc(dump=False, reps=1, skip=()):
    from contextlib import ExitStack

    import concourse.tile as tile
    from concourse import bacc, mybir
    from concourse.masks import make_identity

    f32 = mybir.dt.float32
    bf16 = mybir.dt.bfloat16
    Identity = mybir.ActivationFunctionType.Identity
    Exp = mybir.ActivationFunctionType.Exp
    mult = mybir.AluOpType.mult
    add = mybir.AluOpType.add

    nc = bacc.Bacc("TRN2", target_bir_lowering=False)

    x_ext = nc.dram_tensor("x", [BL, N, C], f32, kind="ExternalInput")
    qkvw_ext = nc.dram_tensor("qkv_w", [O3, C], f32, kind="ExternalInput")
    qkvb_ext = nc.dram_tensor("qkv_b", [O3], f32, kind="ExternalInput")
    projw_ext = nc.dram_tensor("proj_w", [C, C], f32, kind="ExternalInput")
    projb_ext = nc.dram_tensor("proj_b", [C], f32, kind="ExternalInput")
    out_ext = nc.dram_tensor("out", [BL, N, C], f32, kind="ExternalOutput")
    if dump:
        dxt_ext = nc.dram_tensor("d_xt", [128, 6, N], bf16, kind="ExternalOutput")
        dq_ext = nc.dram_tensor("d_q", [128, 6, N], bf16, kind="ExternalOutput")
        dk_ext = nc.dram_tensor("d_k", [128, 6, N], bf16, kind="ExternalOutput")
        dv_ext = nc.dram_tensor("d_v", [128, 4, H, 65], bf16, kind="ExternalOutput")
        da_ext = nc.dram_tensor("d_at", [128, 6, N], bf16, kind="ExternalOutput")
        dpav_ext = nc.dram_tensor("d_pav", [65, N], f32, kind="ExternalOutput")
        drb_ext = nc.dram_tensor("d_rb", [64, N], f32, kind="ExternalOutput")

    # token chunking of the 388 tokens: 128,128,128,4
    TCH = [(0, 128), (128, 128), (256, 128), (384, 4)]

    with tile.TileContext(nc) as tc, ExitStack() as ctx:
        const = ctx.enter_context(tc.tile_pool(name="const", bufs=1))
        stage = ctx.enter_context(tc.tile_pool(name="stage", bufs=4))
        psum = ctx.enter_context(tc.tile_pool(name="ps", bufs=8, space="PSUM"))

        ident = const.tile([128, 128], f32)
        make_identity(nc, ident)

        # ---- weights: PE-transpose fp32, cast to bf16 on the PSUM->SBUF copy
        wT = const.tile([128, 6, O3], bf16)
        projT = const.tile([128, 6, C], bf16)
        for j in range(24):
            wstf = stage.tile([128, C], f32, tag="wstf")
            src = qkvw_ext[j * 128:(j + 1) * 128, :] if j < 18 else \
                projw_ext[(j - 18) * 128:(j - 17) * 128, :]
            nc.sync.dma_start(out=wstf[:], in_=src)
            for ct in range(6):
                pt = psum.tile([128, 128], f32, tag="ps")
                nc.tensor.transpose(pt[:], wstf[:, ct * 128:(ct + 1) * 128], ident[:])
                dst = wT[:, ct, j * 128:(j + 1) * 128] if j < 18 else \
                    projT[:, ct, (j - 18) * 128:(j - 17) * 128]
                if ct % 2 == 0:
                    nc.scalar.copy(out=dst, in_=pt[:])
                else:
                    nc.vector.tensor_copy(out=dst, in_=pt[:])

        # ---- biases ----
        # q/k bias as per-partition fp32 columns [128, 12] (o-tiles 0..11)
        qb_st = stage.tile([12, 128], f32, tag="bst")
        nc.sync.dma_start(out=qb_st[:], in_=qkvb_ext[0:1536].rearrange("(j p) -> j p", p=128))
        pbt = psum.tile([128, 12], f32, tag="ps")
        nc.tensor.transpose(pbt[:], qb_st[:], ident[0:12, 0:12])
        qkb_sb = const.tile([128, 12], f32)
        nc.scalar.copy(out=qkb_sb[:], in_=pbt[:])
        # v bias broadcast along partitions [128, 768]
        vb_bc = const.tile([128, C], f32)
        nc.sync.dma_start(out=vb_bc[:], in_=qkvb_ext[1536:2304].unsqueeze(0).to_broadcast([128, C]))
        # proj bias broadcast
        pb_bc = const.tile([128, C], f32)
        nc.sync.dma_start(out=pb_bc[:], in_=projb_ext[:].unsqueeze(0).to_broadcast([128, C]))

        # ---- per-batch pools ----
        xpool = ctx.enter_context(tc.tile_pool(name="xp", bufs=2))
        xtpool = ctx.enter_context(tc.tile_pool(name="xtp", bufs=2))
        qkpool = ctx.enter_context(tc.tile_pool(name="qkp", bufs=2))
        vpool = ctx.enter_context(tc.tile_pool(name="vp", bufs=2))
        apool = ctx.enter_context(tc.tile_pool(name="ap", bufs=2))
        ppool = ctx.enter_context(tc.tile_pool(name="pp", bufs=3))
        spool = ctx.enter_context(tc.tile_pool(name="ssp", bufs=3))
        opool = ctx.enter_context(tc.tile_pool(name="op", bufs=3))

        def emit_xload(b):
            xf = xpool.tile([128, 4, C], f32, tag="xf")
            nc.sync.dma_start(
                out=xf[:, 0:3, :],
                in_=x_ext[b, 0:384, :].rearrange("(t p) c -> p t c", p=128),
            )
            nc.sync.dma_start(out=xf[0:4, 3, :], in_=x_ext[b, 384:388, :])
            return xf

        def stage1(b, xf, st):
            """Generator: transposes (4 items), q/k groups (12), v halves (8).
            Yields between PE-work units so attention of the previous batch
            can interleave. Fills `st` with the batch's tiles."""
            xTb = xtpool.tile([128, 6, N], bf16, tag="xt")
            st["xT"] = xTb
            for ti, (t0, tp) in enumerate(TCH):
                for ct in range(6):
                    pt = psum.tile([128, 128], f32, tag="ps")
                    nc.tensor.transpose(pt[:, 0:tp], xf[0:tp, ti, ct * 128:(ct + 1) * 128],
                                        ident[0:tp, 0:tp])
                    dst = xTb[:, ct, t0:t0 + tp]
                    if ct % 2 == 0:
                        nc.scalar.copy(out=dst, in_=pt[:, 0:tp])
                    else:
                        nc.vector.tensor_copy(out=dst, in_=pt[:, 0:tp])
                yield

            qTb = qkpool.tile([128, 6, N], bf16, tag="q")
            kTb = qkpool.tile([128, 6, N], bf16, tag="k")
            st["q"], st["k"] = qTb, kTb
            if "qkv" in skip:
                nc.vector.memset(qTb[:, 0:1, 0:2], 0.0)
                nc.vector.memset(kTb[:, 0:1, 0:2], 0.0)
            for j in range(12 if "qkv" not in skip else 0):
                ps = psum.tile([128, N], f32, tag="ps")
                for ct in range(6):
                    nc.tensor.matmul(
                        ps[:],
                        lhsT=wT[:, ct, j * 128:(j + 1) * 128],
                        rhs=xTb[:, ct, :],
                        start=(ct == 0), stop=(ct == 5),
                    )
                dst = qTb[:, j, :] if j < 6 else kTb[:, j - 6, :]
                nc.scalar.activation(out=dst, in_=ps[:], func=Identity,
                                     bias=qkb_sb[:, j:j + 1], scale=1.0)
                yield

            vb = vpool.tile([128, 4, H, 65], bf16, tag="v")
            st["v"] = vb
            nc.vector.memset(vb[:, :, :, 64:65], 1.0)
            for ti, (t0, tp) in enumerate(TCH if "qkv" not in skip else []):
                for o0, on, hs, he in ((0, 512, 0, 8), (512, 256, 8, 12)):
                    pv = psum.tile([128, on], f32, tag="ps")
                    for ct in range(6):
                        nc.tensor.matmul(
                            pv[0:tp, 0:on],
                            lhsT=xTb[:, ct, t0:t0 + tp],
                            rhs=wT[:, ct, 1536 + o0:1536 + o0 + on],
                            start=(ct == 0), stop=(ct == 5),
                        )
                    nc.vector.tensor_tensor(
                        out=vb[0:tp, ti, hs:he, 0:64],
                        in0=pv[0:tp, :].rearrange("p (h d) -> p h d", h=he - hs),
                        in1=vb_bc[0:tp, o0:o0 + on].rearrange("p (h d) -> p h d", h=he - hs),
                        op=add,
                    )
                    yield

        def emit_attention(b, st, filler):
            """Attention heads; pulls filler items between scores and AVs."""
            qTb, kTb, vb = st["q"], st["k"], st["v"]
            xattnT = apool.tile([128, 6, N], bf16, tag="xat")
            st["at"] = xattnT
            if "attn" in skip:
                for _ct in range(6):
                    nc.vector.tensor_copy(out=xattnT[:, _ct, :], in_=wT[:, 0, 0:N])
            nheads = H if "attn" not in skip else 0
            pulled = 0
            for h in range(nheads):
                cth, r0 = h // 2, (h % 2) * 64
                qh = qTb[r0:r0 + 64, cth, :]   # [64, 388] bf16
                kh = kTb[r0:r0 + 64, cth, :]

                # all scores matmuls first; chunk 0 covers ALL queries
                # (template cols 0:128 + search 128:388)
                probs = []
                for kc, (t0, tp) in enumerate(TCH):
                    pss = psum.tile([128, N if kc == 0 else LS], f32, tag="ps")
                    rhs_q = qh[:] if kc == 0 else qh[:, LT:N]
                    nc.tensor.matmul(pss[0:tp, :], lhsT=kh[:, t0:t0 + tp],
                                     rhs=rhs_q, start=True, stop=True)
                    prs = ppool.tile([128, N if kc == 0 else LS], bf16,
                                     tag="pr0" if kc == 0 else "prs")
                    nc.scalar.activation(out=prs[0:tp, :], in_=pss[0:tp, :],
                                         func=Exp, scale=SCALE)
                    probs.append(prs)

                # filler work for neighbouring batches rides in the exp window
                want = (h + 1) * 30 // nheads
                while pulled < want and next(filler, "END") != "END":
                    pulled += 1

                pav = psum.tile([65, N], f32, tag="ps")
                nc.tensor.matmul(pav[:, 0:LT], lhsT=vb[:, 0, h, :],
                                 rhs=probs[0][:, 0:LT], start=True, stop=True)
                nc.tensor.matmul(pav[:, LT:N], lhsT=vb[:, 0, h, :],
                                 rhs=probs[0][:, LT:N], start=True, stop=False)
                for kc, (t0, tp) in list(enumerate(TCH))[1:]:
                    nc.tensor.matmul(pav[:, LT:N], lhsT=vb[0:tp, kc, h, :],
                                     rhs=probs[kc][0:tp, :],
                                     start=False, stop=(kc == 3))

                if dump and b == 0 and h == 0:
                    pavf = spool.tile([65, N], f32, tag="pavf")
                    nc.vector.tensor_copy(out=pavf[:], in_=pav[:])
                    nc.sync.dma_start(out=dpav_ext[:], in_=pavf[:])
                rinv = spool.tile([1, N], f32, tag="ri")
                nc.vector.reciprocal(out=rinv[:], in_=pav[64:65, :])
                rb = spool.tile([64, N], f32, tag="rb")
                nc.gpsimd.partition_broadcast(rb[:], rinv[:])
                if dump and b == 0 and h == 0:
                    nc.sync.dma_start(out=drb_ext[:], in_=rb[:])
                nc.vector.tensor_tensor(out=xattnT[r0:r0 + 64, cth, :],
                                        in0=pav[0:64, :], in1=rb[:], op=mult)

            if dump and b == 0:
                nc.sync.dma_start(out=dxt_ext[:], in_=st["xT"][:, :, 0:N])
                nc.sync.dma_start(out=dq_ext[:], in_=qTb[:])
                nc.sync.dma_start(out=dk_ext[:], in_=kTb[:])
                nc.sync.dma_start(out=dv_ext[:], in_=vb[:])
                nc.sync.dma_start(out=da_ext[:], in_=xattnT[:])
            # drain any remaining filler
            while next(filler, "END") != "END":
                pass

        def proj_gen(b, st):
            """Generator: 4 proj+store chunk items."""
            xattnT = st["at"]
            for ti, (t0, tp) in enumerate(TCH if "proj" not in skip else []):
                ppa = psum.tile([128, 512], f32, tag="ps")
                ppb = psum.tile([128, 256], f32, tag="ps")
                for o0, on, pp in ((0, 512, ppa), (512, 256, ppb)):
                    for ct in range(6):
                        nc.tensor.matmul(
                            pp[0:tp, 0:on],
                            lhsT=xattnT[:, ct, t0:t0 + tp],
                            rhs=projT[:, ct, o0:o0 + on],
                            start=(ct == 0), stop=(ct == 5),
                        )
                osb = opool.tile([128, C], f32, tag="ob")
                nc.vector.tensor_tensor(out=osb[0:tp, 0:512], in0=ppa[0:tp, :],
                                        in1=pb_bc[0:tp, 0:512], op=add)
                nc.vector.tensor_tensor(out=osb[0:tp, 512:768], in0=ppb[0:tp, :],
                                        in1=pb_bc[0:tp, 512:768], op=add)
                nc.sync.dma_start(out=out_ext[b, t0:t0 + tp, :], in_=osb[0:tp, :])
                yield

        # ---- software-pipelined batch loop ----
        from itertools import chain

        seq = [bb for _ in range(reps) for bb in range(BL)]
        states = [dict() for _ in seq]
        xf0 = emit_xload(seq[0])
        for _ in stage1(seq[0], xf0, states[0]):
            pass
        prev_proj = iter(())
        for i, b in enumerate(seq):
            if i + 1 < len(seq):
                xf_n = emit_xload(seq[i + 1])
                nxt = stage1(seq[i + 1], xf_n, states[i + 1])
            else:
                nxt = iter(())
            emit_attention(b, states[i], chain(prev_proj, nxt))
            prev_proj = proj_gen(b, states[i])
        for _ in prev_proj:
            pass

    nc.compile()
    return nc


def _get_nc():
    if "nc" not in _NC_CACHE:
        _NC_CACHE["nc"] = _build_nc()
    return _NC_CACHE["nc"]


def kernel(x, qkv_w, qkv_b, proj_w, proj_b, t_h=8, t_w=8, s_h=16, s_w=16):
    from concourse.bass_utils import run_bass_kernel_spmd

    x = np.ascontiguousarray(np.asarray(x, dtype=np.float32))
    qkv_w = np.ascontiguousarray(np.asarray(qkv_w, dtype=np.float32))
    qkv_b = np.ascontiguousarray(np.asarray(qkv_b, dtype=np.float32))
    proj_w = np.ascontiguousarray(np.asarray(proj_w, dtype=np.float32))
    proj_b = np.ascontiguousarray(np.asarray(proj_b, dtype=np.float32))

    nc = _get_nc()
    in_maps = [
        {
            "x": x[i * BL:(i + 1) * BL],
            "qkv_w": qkv_w,
            "qkv_b": qkv_b,
            "proj_w": proj_w,
            "proj_b": proj_b,
        }
        for i in range(NCORES)
    ]
    res = run_bass_kernel_spmd(nc, in_maps, core_ids=list(range(NCORES)))
    out = np.concatenate([res.results[i]["out"] for i in range(NCORES)], axis=0)
    return out.astype(np.float32)


# revision 18
# speedup vs baseline: 12114.2990x; 1.1364x over previous
"""Sparse-attention Trainium2 kernel, 8-way data-parallel over batch.

Reference computation (per batch):
  qkv = x @ qkv_w.T + qkv_b              -> split q,k,v [H=12, N=388, D=64]
  template queries (tokens 0:128) attend to template keys (0:128)
  search queries (tokens 128:388) attend to all 388 keys
  out = concat @ proj_w.T + proj_b

Kernel strategy per core (B_local=8 batches, all compute on device, bf16
matmuls with fp32 PSUM accumulation):
  - x cast to bf16, transposed feature-major via DMA-xbar transposes.
  - q^T,k^T = W^T-stationary matmuls (feature-major out, per-partition bias
    added in fp32 on ACT during the PSUM->SBUF copy).
  - v = x^T-stationary matmuls (token-major out), stored per-head with a ones
    column appended so the attention-value matmul also produces softmax sums.
  - scores computed TRANSPOSED: S^T[k,q] = k^T-slices as lhsT, q^T as rhs.
    exp on ACT (scale=1/8 folded in), probs in bf16.
  - AV: out^T[d,q] accumulated over k-chunks; row 64 = softmax denominators.
  - normalize: reciprocal on DVE, partition_broadcast on GpSimd, multiply on
    DVE (writes bf16 feature-major attention output).
  - proj matmul reads attention output directly (no transposes), bias on DVE,
    DMA out token-major fp32.
"""

import numpy as np

B, N, C = 64, 388, 768
H, D = 12, 64
LT = 128          # template tokens (= first token chunk, exactly)
LS = N - LT       # 260 search tokens
NCORES = 8
BL = B // NCORES  # 8 batches per core
O3 = 3 * C        # 2304
SCALE = 0.125
NPAD = 416        # 388 tokens padded to 32-multiple for DMA-transpose tiles

_NC_CACHE = {}


def _build_nc(dump=False, reps=1, skip=()):
    from contextlib import ExitStack

    import concourse.tile as tile
    from concourse import bacc, mybir
    from concourse.masks import make_identity

    f32 = mybir.dt.float32
    bf16 = mybir.dt.bfloat16
    Identity = mybir.ActivationFunctionType.Identity
    Exp = mybir.ActivationFunctionType.Exp
    mult = mybir.AluOpType.mult
    add = mybir.AluOpType.add

    nc = bacc.Bacc("TRN2", target_bir_lowering=False)

    x_ext = nc.dram_tensor("x", [BL, N, C], f32, kind="ExternalInput")
    qkvw_ext = nc.dram_tensor("qkv_w", [O3, C], f32, kind="ExternalInput")
    qkvb_ext = nc.dram_tensor("qkv_b", [O3], f32, kind="ExternalInput")
    projw_ext = nc.dram_tensor("proj_w", [C, C], f32, kind="ExternalInput")
    projb_ext = nc.dram_tensor("proj_b", [C], f32, kind="ExternalInput")
    out_ext = nc.dram_tensor("out", [BL, N, C], f32, kind="ExternalOutput")
    if dump:
        dxt_ext = nc.dram_tensor("d_xt", [128, 6, N], bf16, kind="ExternalOutput")
        dq_ext = nc.dram_tensor("d_q", [128, 6, N], bf16, kind="ExternalOutput")
        dk_ext = nc.dram_tensor("d_k", [128, 6, N], bf16, kind="ExternalOutput")
        dv_ext = nc.dram_tensor("d_v", [128, 4, H, 65], bf16, kind="ExternalOutput")
        da_ext = nc.dram_tensor("d_at", [128, 6, N], bf16, kind="ExternalOutput")
        dpav_ext = nc.dram_tensor("d_pav", [65, N], f32, kind="ExternalOutput")
        drb_ext = nc.dram_tensor("d_rb", [64, N], f32, kind="ExternalOutput")

    # token chunking of the 388 tokens: 128,128,128,4
    TCH = [(0, 128), (128, 128), (256, 128), (384, 4)]

    with tile.TileContext(nc) as tc, ExitStack() as ctx:
        const = ctx.enter_context(tc.tile_pool(name="const", bufs=1))
        stage = ctx.enter_context(tc.tile_pool(name="stage", bufs=4))
        psum = ctx.enter_context(tc.tile_pool(name="ps", bufs=8, space="PSUM"))

        ident = const.tile([128, 128], f32)
        make_identity(nc, ident)

        # ---- weights: PE-transpose fp32, cast to bf16 on the PSUM->SBUF copy
        wT = const.tile([128, 6, O3], bf16)
        projT = const.tile([128, 6, C], bf16)
        for j in range(24):
            wstf = stage.tile([128, C], f32, tag="wstf")
            src = qkvw_ext[j * 128:(j + 1) * 128, :] if j < 18 else \
                projw_ext[(j - 18) * 128:(j - 17) * 128, :]
            nc.sync.dma_start(out=wstf[:], in_=src)
            for ct in range(6):
                pt = psum.tile([128, 128], f32, tag="ps")
                nc.tensor.transpose(pt[:], wstf[:, ct * 128:(ct + 1) * 128], ident[:])
                dst = wT[:, ct, j * 128:(j + 1) * 128] if j < 18 else \
                    projT[:, ct, (j - 18) * 128:(j - 17) * 128]
                if ct % 2 == 0:
                    nc.scalar.copy(out=dst, in_=pt[:])
                else:
                    nc.vector.tensor_copy(out=dst, in_=pt[:])

        # ---- biases ----
        # q/k bias as per-partition fp32 columns [128, 12] (o-tiles 0..11)
        qb_st = stage.tile([12, 128], f32, tag="bst")
        nc.sync.dma_start(out=qb_st[:], in_=qkvb_ext[0:1536].rearrange("(j p) -> j p", p=128))
        pbt = psum.tile([128, 12], f32, tag="ps")
        nc.tensor.transpose(pbt[:], qb_st[:], ident[0:12, 0:12])
        qkb_sb = const.tile([128, 12], f32)
        nc.scalar.copy(out=qkb_sb[:], in_=pbt[:])
        # v bias broadcast along partitions [128, 768]
        vb_bc = const.tile([128, C], f32)
        nc.sync.dma_start(out=vb_bc[:], in_=qkvb_ext[1536:2304].unsqueeze(0).to_broadcast([128, C]))
        # proj bias broadcast
        pb_bc = const.tile([128, C], f32)
        nc.sync.dma_start(out=pb_bc[:], in_=projb_ext[:].unsqueeze(0).to_broadcast([128, C]))

        # ---- per-batch pools ----
        xpool = ctx.enter_context(tc.tile_pool(name="xp", bufs=2))
        xtpool = ctx.enter_context(tc.tile_pool(name="xtp", bufs=2))
        qkpool = ctx.enter_context(tc.tile_pool(name="qkp", bufs=2))
        vpool = ctx.enter_context(tc.tile_pool(name="vp", bufs=2))
        apool = ctx.enter_context(tc.tile_pool(name="ap", bufs=2))
        ppool = ctx.enter_context(tc.tile_pool(name="pp", bufs=3))
        spool = ctx.enter_context(tc.tile_pool(name="ssp", bufs=3))
        opool = ctx.enter_context(tc.tile_pool(name="op", bufs=3))

        def emit_xload(b):
            xf = xpool.tile([128, 4, C], f32, tag="xf")
            nc.sync.dma_start(
                out=xf[:, 0:3, :],
                in_=x_ext[b, 0:384, :].rearrange("(t p) c -> p t c", p=128),
            )
            nc.sync.dma_start(out=xf[0:4, 3, :], in_=x_ext[b, 384:388, :])
            return xf

        def stage1(b, xf, st):
            """Generator: transposes (4 items), q/k groups (12), v halves (8).
            Yields between PE-work units so attention of the previous batch
            can interleave. Fills `st` with the batch's tiles."""
            xTb = xtpool.tile([128, 6, N], bf16, tag="xt")
            st["xT"] = xTb
            for ti, (t0, tp) in enumerate(TCH):
                for ct in range(6):
                    pt = psum.tile([128, 128], f32, tag="ps")
                    nc.tensor.transpose(pt[:, 0:tp], xf[0:tp, ti, ct * 128:(ct + 1) * 128],
                                        ident[0:tp, 0:tp])
                    dst = xTb[:, ct, t0:t0 + tp]
                    if ct % 2 == 0:
                        nc.scalar.copy(out=dst, in_=pt[:, 0:tp])
                    else:
                        nc.vector.tensor_copy(out=dst, in_=pt[:, 0:tp])
                yield

            qTb = qkpool.tile([128, 6, N], bf16, tag="q")
            kTb = qkpool.tile([128, 6, N], bf16, tag="k")
            st["q"], st["k"] = qTb, kTb
            if "qkv" in skip:
                nc.vector.memset(qTb[:, 0:1, 0:2], 0.0)
                nc.vector.memset(kTb[:, 0:1, 0:2], 0.0)
            for j in range(12 if "qkv" not in skip else 0):
                ps = psum.tile([128, N], f32, tag="ps")
                for ct in range(6):
                    nc.tensor.matmul(
                        ps[:],
                        lhsT=wT[:, ct, j * 128:(j + 1) * 128],
                        rhs=xTb[:, ct, :],
                        start=(ct == 0), stop=(ct == 5),
                    )
                dst = qTb[:, j, :] if j < 6 else kTb[:, j - 6, :]
                nc.scalar.activation(out=dst, in_=ps[:], func=Identity,
                                     bias=qkb_sb[:, j:j + 1], scale=1.0)
                yield

            vb = vpool.tile([128, 4, H, 65], bf16, tag="v")
            st["v"] = vb
            nc.vector.memset(vb[:, :, :, 64:65], 1.0)
            for ti, (t0, tp) in enumerate(TCH if "qkv" not in skip else []):
                for o0, on, hs, he in ((0, 512, 0, 8), (512, 256, 8, 12)):
                    pv = psum.tile([128, on], f32, tag="ps")
                    for ct in range(6):
                        nc.tensor.matmul(
                            pv[0:tp, 0:on],
                            lhsT=xTb[:, ct, t0:t0 + tp],
                            rhs=wT[:, ct, 1536 + o0:1536 + o0 + on],
                            start=(ct == 0), stop=(ct == 5),
                        )
                    nc.vector.tensor_tensor(
                        out=vb[0:tp, ti, hs:he, 0:64],
                        in0=pv[0:tp, :].rearrange("p (h d) -> p h d", h=he - hs),
                        in1=vb_bc[0:tp, o0:o0 + on].rearrange("p (h d) -> p h d", h=he - hs),
                        op=add,
                    )
                    yield

        def emit_attention(b, st, filler):
            """Attention heads; pulls filler items between scores and AVs."""
            qTb, kTb, vb = st["q"], st["k"], st["v"]
            xattnT = apool.tile([128, 6, N], bf16, tag="xat")
            st["at"] = xattnT
            if "attn" in skip:
                for _ct in range(6):
                    nc.vector.tensor_copy(out=xattnT[:, _ct, :], in_=wT[:, 0, 0:N])
            nheads = H if "attn" not in skip else 0
            pulled = 0
            for h in range(nheads):
                cth, r0 = h // 2, (h % 2) * 64
                qh = qTb[r0:r0 + 64, cth, :]   # [64, 388] bf16
                kh = kTb[r0:r0 + 64, cth, :]

                # all scores matmuls first; chunk 0 covers ALL queries
                # (template cols 0:128 + search 128:388)
                probs = []
                for kc, (t0, tp) in enumerate(TCH):
                    pss = psum.tile([128, N if kc == 0 else LS], f32, tag="ps")
                    rhs_q = qh[:] if kc == 0 else qh[:, LT:N]
                    nc.tensor.matmul(pss[0:tp, :], lhsT=kh[:, t0:t0 + tp],
                                     rhs=rhs_q, start=True, stop=True)
                    prs = ppool.tile([128, N if kc == 0 else LS], bf16,
                                     tag="pr0" if kc == 0 else "prs")
                    nc.scalar.activation(out=prs[0:tp, :], in_=pss[0:tp, :],
                                         func=Exp, scale=SCALE)
                    probs.append(prs)

                # filler work for neighbouring batches rides in the exp window
                want = (h + 1) * 30 // nheads
                while pulled < want and next(filler, "END") != "END":
                    pulled += 1

                pav = psum.tile([65, N], f32, tag="ps")
                nc.tensor.matmul(pav[:, 0:N], lhsT=vb[:, 0, h, :],
                                 rhs=probs[0][:, 0:N], start=True, stop=False)
                for kc, (t0, tp) in list(enumerate(TCH))[1:]:
                    nc.tensor.matmul(pav[:, LT:N], lhsT=vb[0:tp, kc, h, :],
                                     rhs=probs[kc][0:tp, :],
                                     start=False, stop=(kc == 3))

                if dump and b == 0 and h == 0:
                    pavf = spool.tile([65, N], f32, tag="pavf")
                    nc.vector.tensor_copy(out=pavf[:], in_=pav[:])
                    nc.sync.dma_start(out=dpav_ext[:], in_=pavf[:])
                rinv = spool.tile([1, N], f32, tag="ri")
                nc.vector.reciprocal(out=rinv[:], in_=pav[64:65, :])
                rb = spool.tile([64, N], f32, tag="rb")
                nc.gpsimd.partition_broadcast(rb[:], rinv[:])
                if dump and b == 0 and h == 0:
                    nc.sync.dma_start(out=drb_ext[:], in_=rb[:])
                nc.vector.tensor_tensor(out=xattnT[r0:r0 + 64, cth, :],
                                        in0=pav[0:64, :], in1=rb[:], op=mult)

            if dump and b == 0:
                nc.sync.dma_start(out=dxt_ext[:], in_=st["xT"][:, :, 0:N])
                nc.sync.dma_start(out=dq_ext[:], in_=qTb[:])
                nc.sync.dma_start(out=dk_ext[:], in_=kTb[:])
                nc.sync.dma_start(out=dv_ext[:], in_=vb[:])
                nc.sync.dma_start(out=da_ext[:], in_=xattnT[:])
            # drain any remaining filler
            while next(filler, "END") != "END":
                pass

        def proj_gen(b, st):
            """Generator: 4 proj+store chunk items."""
            xattnT = st["at"]
            for ti, (t0, tp) in enumerate(TCH if "proj" not in skip else []):
                ppa = psum.tile([128, 512], f32, tag="ps")
                ppb = psum.tile([128, 256], f32, tag="ps")
                for o0, on, pp in ((0, 512, ppa), (512, 256, ppb)):
                    for ct in range(6):
                        nc.tensor.matmul(
                            pp[0:tp, 0:on],
                            lhsT=xattnT[:, ct, t0:t0 + tp],
                            rhs=projT[:, ct, o0:o0 + on],
                            start=(ct == 0), stop=(ct == 5),
                        )
                osb = opool.tile([128, C], f32, tag="ob")
                nc.vector.tensor_tensor(out=osb[0:tp, 0:512], in0=ppa[0:tp, :],
                                        in1=pb_bc[0:tp, 0:512], op=add)
                nc.vector.tensor_tensor(out=osb[0:tp, 512:768], in0=ppb[0:tp, :],
                                        in1=pb_bc[0:tp, 512:768], op=add)
                nc.sync.dma_start(out=out_ext[b, t0:t0 + tp, :], in_=osb[0:tp, :])
                yield

        # ---- software-pipelined batch loop ----
        from itertools import chain

        seq = [bb for _ in range(reps) for bb in range(BL)]
        states = [dict() for _ in seq]
        xf0 = emit_xload(seq[0])
        for _ in stage1(seq[0], xf0, states[0]):
            pass
        prev_proj = iter(())
        for i, b in enumerate(seq):
            if i + 1 < len(seq):
                xf_n = emit_xload(seq[i + 1])
                nxt = stage1(seq[i + 1], xf_n, states[i + 1])
            else:
                nxt = iter(())
            emit_attention(b, states[i], chain(prev_proj, nxt))
            prev_proj = proj_gen(b, states[i])
        for _ in prev_proj:
            pass

    nc.compile()
    return nc


def _get_nc():
    if "nc" not in _NC_CACHE:
        _NC_CACHE["nc"] = _build_nc()
    return _NC_CACHE["nc"]


def kernel(x, qkv_w, qkv_b, proj_w, proj_b, t_h=8, t_w=8, s_h=16, s_w=16):
    from concourse.bass_utils import run_bass_kernel_spmd

    x = np.ascontiguousarray(np.asarray(x, dtype=np.float32))
    qkv_w = np.ascontiguousarray(np.asarray(qkv_w, dtype=np.float32))
    qkv_b = np.ascontiguousarray(np.asarray(qkv_b, dtype=np.float32))
    proj_w = np.ascontiguousarray(np.asarray(proj_w, dtype=np.float32))
    proj_b = np.ascontiguousarray(np.asarray(proj_b, dtype=np.float32))

    nc = _get_nc()
    in_maps = [
        {
            "x": x[i * BL:(i + 1) * BL],
            "qkv_w": qkv_w,
            "qkv_b": qkv_b,
            "proj_w": proj_w,
            "proj_b": proj_b,
        }
        for i in range(NCORES)
    ]
    res = run_bass_kernel_spmd(nc, in_maps, core_ids=list(range(NCORES)))
    out = np.concatenate([res.results[i]["out"] for i in range(NCORES)], axis=0)
    return out.astype(np.float32)


# revision 24
# speedup vs baseline: 13247.0682x; 1.0935x over previous
"""Sparse-attention Trainium2 kernel, 8-way data-parallel over batch.

Reference computation (per batch):
  qkv = x @ qkv_w.T + qkv_b              -> split q,k,v [H=12, N=388, D=64]
  template queries (tokens 0:128) attend to template keys (0:128)
  search queries (tokens 128:388) attend to all 388 keys
  out = concat @ proj_w.T + proj_b

Kernel strategy per core (B_local=8 batches, all compute on device, bf16
matmuls with fp32 PSUM accumulation):
  - x cast to bf16, transposed feature-major via DMA-xbar transposes.
  - q^T,k^T = W^T-stationary matmuls (feature-major out, per-partition bias
    added in fp32 on ACT during the PSUM->SBUF copy).
  - v = x^T-stationary matmuls (token-major out), stored per-head with a ones
    column appended so the attention-value matmul also produces softmax sums.
  - scores computed TRANSPOSED: S^T[k,q] = k^T-slices as lhsT, q^T as rhs.
    exp on ACT (scale=1/8 folded in), probs in bf16.
  - AV: out^T[d,q] accumulated over k-chunks; row 64 = softmax denominators.
  - normalize: reciprocal on DVE, partition_broadcast on GpSimd, multiply on
    DVE (writes bf16 feature-major attention output).
  - proj matmul reads attention output directly (no transposes), bias on DVE,
    DMA out token-major fp32.
"""

import numpy as np

B, N, C = 64, 388, 768
H, D = 12, 64
LT = 128          # template tokens (= first token chunk, exactly)
LS = N - LT       # 260 search tokens
NCORES = 8
BL = B // NCORES  # 8 batches per core
O3 = 3 * C        # 2304
SCALE = 0.125
NPAD = 416        # 388 tokens padded to 32-multiple for DMA-transpose tiles

_NC_CACHE = {}


def _build_nc(dump=False, reps=1, skip=()):
    from contextlib import ExitStack

    import concourse.tile as tile
    from concourse import bacc, mybir
    from concourse.masks import make_identity

    f32 = mybir.dt.float32
    bf16 = mybir.dt.bfloat16
    Identity = mybir.ActivationFunctionType.Identity
    Exp = mybir.ActivationFunctionType.Exp
    mult = mybir.AluOpType.mult
    add = mybir.AluOpType.add

    nc = bacc.Bacc("TRN2", target_bir_lowering=False)

    x_ext = nc.dram_tensor("x", [BL, N, C], f32, kind="ExternalInput")
    qkvw_ext = nc.dram_tensor("qkv_w", [O3, C], f32, kind="ExternalInput")
    qkvb_ext = nc.dram_tensor("qkv_b", [O3], f32, kind="ExternalInput")
    projw_ext = nc.dram_tensor("proj_w", [C, C], f32, kind="ExternalInput")
    projb_ext = nc.dram_tensor("proj_b", [C], f32, kind="ExternalInput")
    out_ext = nc.dram_tensor("out", [BL, N, C], f32, kind="ExternalOutput")
    if dump:
        dxt_ext = nc.dram_tensor("d_xt", [128, 6, N], bf16, kind="ExternalOutput")
        dq_ext = nc.dram_tensor("d_q", [128, 6, N], bf16, kind="ExternalOutput")
        dk_ext = nc.dram_tensor("d_k", [128, 6, N], bf16, kind="ExternalOutput")
        dv_ext = nc.dram_tensor("d_v", [128, 4, H, 65], bf16, kind="ExternalOutput")
        da_ext = nc.dram_tensor("d_at", [128, 6, N], bf16, kind="ExternalOutput")
        dpav_ext = nc.dram_tensor("d_pav", [65, N], f32, kind="ExternalOutput")
        drb_ext = nc.dram_tensor("d_rb", [64, N], f32, kind="ExternalOutput")

    # token chunking of the 388 tokens: 128,128,128,4
    TCH = [(0, 128), (128, 128), (256, 128), (384, 4)]

    with tile.TileContext(nc) as tc, ExitStack() as ctx:
        const = ctx.enter_context(tc.tile_pool(name="const", bufs=1))
        stage = ctx.enter_context(tc.tile_pool(name="stage", bufs=4))
        psum = ctx.enter_context(tc.tile_pool(name="ps", bufs=8, space="PSUM"))

        ident = const.tile([128, 128], f32)
        make_identity(nc, ident)

        # ---- weights/biases declared here; emission interleaved with batch 0
        wT = const.tile([128, 6, O3], bf16)
        projT = const.tile([128, 6, C], bf16)
        qkb_sb = const.tile([128, 12], f32)
        vb_bc = const.tile([128, C], f32)
        pb_bc = const.tile([128, C], f32)

        def weights_gen():
            # biases first (tiny)
            qb_st = stage.tile([12, 128], f32, tag="bst")
            nc.sync.dma_start(out=qb_st[:], in_=qkvb_ext[0:1536].rearrange("(j p) -> j p", p=128))
            pbt = psum.tile([128, 12], f32, tag="ps")
            nc.tensor.transpose(pbt[:], qb_st[:], ident[0:12, 0:12])
            nc.scalar.copy(out=qkb_sb[:], in_=pbt[:])
            nc.sync.dma_start(out=vb_bc[:], in_=qkvb_ext[1536:2304].unsqueeze(0).to_broadcast([128, C]))
            nc.sync.dma_start(out=pb_bc[:], in_=projb_ext[:].unsqueeze(0).to_broadcast([128, C]))
            # weight chunks: PE-transpose fp32, cast to bf16 on PSUM->SBUF copy
            for j in range(24):
                wstf = stage.tile([128, C], f32, tag="wstf")
                src = qkvw_ext[j * 128:(j + 1) * 128, :] if j < 18 else \
                    projw_ext[(j - 18) * 128:(j - 17) * 128, :]
                nc.sync.dma_start(out=wstf[:], in_=src)
                for ct in range(6):
                    pt = psum.tile([128, 128], f32, tag="ps")
                    nc.tensor.transpose(pt[:], wstf[:, ct * 128:(ct + 1) * 128], ident[:])
                    dst = wT[:, ct, j * 128:(j + 1) * 128] if j < 18 else \
                        projT[:, ct, (j - 18) * 128:(j - 17) * 128]
                    if ct % 2 == 0:
                        nc.scalar.copy(out=dst, in_=pt[:])
                    else:
                        nc.vector.tensor_copy(out=dst, in_=pt[:])
                yield

        # ---- per-batch pools ----
        xpool = ctx.enter_context(tc.tile_pool(name="xp", bufs=2))
        xtpool = ctx.enter_context(tc.tile_pool(name="xtp", bufs=2))
        qkpool = ctx.enter_context(tc.tile_pool(name="qkp", bufs=2))
        vpool = ctx.enter_context(tc.tile_pool(name="vp", bufs=2))
        apool = ctx.enter_context(tc.tile_pool(name="ap", bufs=2))
        ppool = ctx.enter_context(tc.tile_pool(name="pp", bufs=3))
        spool = ctx.enter_context(tc.tile_pool(name="ssp", bufs=3))
        opool = ctx.enter_context(tc.tile_pool(name="op", bufs=3))

        def emit_xload(b):
            xf = xpool.tile([128, 4, C], f32, tag="xf")
            nc.sync.dma_start(
                out=xf[:, 0:3, :],
                in_=x_ext[b, 0:384, :].rearrange("(t p) c -> p t c", p=128),
            )
            nc.sync.dma_start(out=xf[0:4, 3, :], in_=x_ext[b, 384:388, :])
            return xf

        def stage1(b, xf, st):
            """Generator: transposes (4 items), q/k groups (12), v halves (8).
            Yields between PE-work units so attention of the previous batch
            can interleave. Fills `st` with the batch's tiles."""
            xTb = xtpool.tile([128, 6, N], bf16, tag="xt")
            st["xT"] = xTb
            for ti, (t0, tp) in enumerate(TCH):
                for ct in range(6):
                    pt = psum.tile([128, 128], f32, tag="ps")
                    nc.tensor.transpose(pt[:, 0:tp], xf[0:tp, ti, ct * 128:(ct + 1) * 128],
                                        ident[0:tp, 0:tp])
                    dst = xTb[:, ct, t0:t0 + tp]
                    if ct % 2 == 0:
                        nc.scalar.copy(out=dst, in_=pt[:, 0:tp])
                    else:
                        nc.vector.tensor_copy(out=dst, in_=pt[:, 0:tp])
                yield

            qTb = qkpool.tile([128, 6, N], bf16, tag="q")
            kTb = qkpool.tile([128, 6, N], bf16, tag="k")
            st["q"], st["k"] = qTb, kTb
            if "qkv" in skip:
                nc.vector.memset(qTb[:, 0:1, 0:2], 0.0)
                nc.vector.memset(kTb[:, 0:1, 0:2], 0.0)
            for j in range(12 if "qkv" not in skip else 0):
                ps = psum.tile([128, N], f32, tag="ps")
                for ct in range(6):
                    nc.tensor.matmul(
                        ps[:],
                        lhsT=wT[:, ct, j * 128:(j + 1) * 128],
                        rhs=xTb[:, ct, :],
                        start=(ct == 0), stop=(ct == 5),
                    )
                dst = qTb[:, j, :] if j < 6 else kTb[:, j - 6, :]
                nc.scalar.activation(out=dst, in_=ps[:], func=Identity,
                                     bias=qkb_sb[:, j:j + 1], scale=1.0)
                yield

            # block-diagonal remainder tiles for the 4 leftover key tokens:
            # kTrem[:, cth, 0:4] = even head's k-remainder (d-rows 0:64),
            # kTrem[:, cth, 32:36] = odd head's (d-rows 64:128); other columns
            # zero so one matmul yields both heads' remainder scores.
            kTrem = qkpool.tile([128, 6, 36], bf16, tag="krem")
            st["krem"] = kTrem
            nc.vector.memset(kTrem[:], 0.0)
            nc.vector.tensor_copy(out=kTrem[0:64, :, 0:4], in_=kTb[0:64, :, 384:388])
            nc.vector.tensor_copy(out=kTrem[64:128, :, 32:36], in_=kTb[64:128, :, 384:388])

            vb = vpool.tile([128, 4, H, 65], bf16, tag="v")
            st["v"] = vb
            nc.vector.memset(vb[:, :, :, 64:65], 1.0)
            for ti, (t0, tp) in enumerate(TCH if "qkv" not in skip else []):
                for o0, on, hs, he in ((0, 512, 0, 8), (512, 256, 8, 12)):
                    pv = psum.tile([128, on], f32, tag="ps")
                    for ct in range(6):
                        nc.tensor.matmul(
                            pv[0:tp, 0:on],
                            lhsT=xTb[:, ct, t0:t0 + tp],
                            rhs=wT[:, ct, 1536 + o0:1536 + o0 + on],
                            start=(ct == 0), stop=(ct == 5),
                        )
                    nc.vector.tensor_tensor(
                        out=vb[0:tp, ti, hs:he, 0:64],
                        in0=pv[0:tp, :].rearrange("p (h d) -> p h d", h=he - hs),
                        in1=vb_bc[0:tp, o0:o0 + on].rearrange("p (h d) -> p h d", h=he - hs),
                        op=add,
                    )
                    yield

            # v-remainder regrouped to match kTrem's partition layout
            vrem = vpool.tile([36, 6, 65], bf16, tag="vrem")
            st["vrem"] = vrem
            if "qkv" not in skip:
                nc.vector.tensor_copy(out=vrem[0:4, :, :], in_=vb[0:4, 3, 0:12:2, :])
                nc.vector.tensor_copy(out=vrem[32:36, :, :], in_=vb[0:4, 3, 1:12:2, :])

        def emit_attention(b, st, filler):
            """Attention heads; pulls filler items between scores and AVs."""
            qTb, kTb, vb = st["q"], st["k"], st["v"]
            kTrem, vrem = st["krem"], st["vrem"]
            xattnT = apool.tile([128, 6, N], bf16, tag="xat")
            st["at"] = xattnT
            if "attn" in skip:
                for _ct in range(6):
                    nc.vector.tensor_copy(out=xattnT[:, _ct, :], in_=wT[:, 0, 0:N])
            nheads = H if "attn" not in skip else 0
            pulled = 0
            for h in range(nheads):
                cth, r0 = h // 2, (h % 2) * 64
                qh = qTb[r0:r0 + 64, cth, :]   # [64, 388] bf16
                kh = kTb[r0:r0 + 64, cth, :]

                # all scores matmuls first; chunk 0 covers ALL queries
                # (template cols 0:128 + search 128:388); chunk 3 (the 4
                # remainder keys) is computed for the HEAD PAIR at even h via
                # the block-diagonal kTrem in one matmul + one exp
                probs = []
                for kc, (t0, tp) in list(enumerate(TCH))[0:3]:
                    pss = psum.tile([128, N if kc == 0 else LS], f32, tag="ps")
                    rhs_q = qh[:] if kc == 0 else qh[:, LT:N]
                    nc.tensor.matmul(pss[0:tp, :], lhsT=kh[:, t0:t0 + tp],
                                     rhs=rhs_q, start=True, stop=True)
                    prs = ppool.tile([128, N if kc == 0 else LS], bf16,
                                     tag="pr0" if kc == 0 else "prs")
                    nc.scalar.activation(out=prs[0:tp, :], in_=pss[0:tp, :],
                                         func=Exp, scale=SCALE)
                    probs.append(prs)
                if h % 2 == 0:
                    psr = psum.tile([36, LS], f32, tag="ps")
                    nc.tensor.matmul(psr[:], lhsT=kTrem[:, cth, :],
                                     rhs=qTb[:, cth, LT:N], start=True, stop=True)
                    prr = ppool.tile([36, LS], bf16, tag="prr")
                    nc.scalar.activation(out=prr[:], in_=psr[:], func=Exp, scale=SCALE)
                    st["prr"] = prr
                else:
                    prr = st["prr"]

                # filler work for neighbouring batches rides in the exp window
                want = (h + 1) * 30 // nheads
                while pulled < want and next(filler, "END") != "END":
                    pulled += 1

                pav = psum.tile([65, N], f32, tag="ps")
                nc.tensor.matmul(pav[:, 0:N], lhsT=vb[:, 0, h, :],
                                 rhs=probs[0][:, 0:N], start=True, stop=False)
                for kc, (t0, tp) in list(enumerate(TCH))[1:3]:
                    nc.tensor.matmul(pav[:, LT:N], lhsT=vb[0:tp, kc, h, :],
                                     rhs=probs[kc][0:tp, :],
                                     start=False, stop=False)
                rr = (h % 2) * 32
                nc.tensor.matmul(pav[:, LT:N], lhsT=vrem[rr:rr + 4, cth, :],
                                 rhs=prr[rr:rr + 4, :], start=False, stop=True)

                if dump and b == 0 and h == 0:
                    pavf = spool.tile([65, N], f32, tag="pavf")
                    nc.vector.tensor_copy(out=pavf[:], in_=pav[:])
                    nc.sync.dma_start(out=dpav_ext[:], in_=pavf[:])
                rinv = spool.tile([1, N], f32, tag="ri")
                nc.vector.reciprocal(out=rinv[:], in_=pav[64:65, :])
                rb = spool.tile([64, N], f32, tag="rb")
                nc.gpsimd.partition_broadcast(rb[:], rinv[:])
                if dump and b == 0 and h == 0:
                    nc.sync.dma_start(out=drb_ext[:], in_=rb[:])
                nc.vector.tensor_tensor(out=xattnT[r0:r0 + 64, cth, :],
                                        in0=pav[0:64, :], in1=rb[:], op=mult)

            if dump and b == 0:
                nc.sync.dma_start(out=dxt_ext[:], in_=st["xT"][:, :, 0:N])
                nc.sync.dma_start(out=dq_ext[:], in_=qTb[:])
                nc.sync.dma_start(out=dk_ext[:], in_=kTb[:])
                nc.sync.dma_start(out=dv_ext[:], in_=vb[:])
                nc.sync.dma_start(out=da_ext[:], in_=xattnT[:])
            # drain any remaining filler
            while next(filler, "END") != "END":
                pass

        def proj_gen(b, st):
            """Generator: 4 proj+store chunk items."""
            xattnT = st["at"]
            for ti, (t0, tp) in enumerate(TCH if "proj" not in skip else []):
                ppa = psum.tile([128, 512], f32, tag="ps")
                ppb = psum.tile([128, 256], f32, tag="ps")
                for o0, on, pp in ((0, 512, ppa), (512, 256, ppb)):
                    for ct in range(6):
                        nc.tensor.matmul(
                            pp[0:tp, 0:on],
                            lhsT=xattnT[:, ct, t0:t0 + tp],
                            rhs=projT[:, ct, o0:o0 + on],
                            start=(ct == 0), stop=(ct == 5),
                        )
                osb = opool.tile([128, C], f32, tag="ob")
                nc.vector.tensor_tensor(out=osb[0:tp, 0:512], in0=ppa[0:tp, :],
                                        in1=pb_bc[0:tp, 0:512], op=add)
                nc.vector.tensor_tensor(out=osb[0:tp, 512:768], in0=ppb[0:tp, :],
                                        in1=pb_bc[0:tp, 512:768], op=add)
                nc.sync.dma_start(out=out_ext[b, t0:t0 + tp, :], in_=osb[0:tp, :])
                yield

        # ---- software-pipelined batch loop ----
        from itertools import chain

        seq = [bb for _ in range(reps) for bb in range(BL)]
        states = [dict() for _ in seq]
        # prologue: interleave the 24 weight-prep chunks with batch 0's
        # stage1 items (transposes need no weights; qk group j needs weight
        # chunk j which is already emitted by then; v needs chunks 12-17)
        gw = weights_gen()
        xf0 = emit_xload(seq[0])
        g0 = stage1(seq[0], xf0, states[0])
        for _ in range(24):
            next(gw, None)
            next(g0, None)
        for _ in g0:
            pass
        prev_proj = iter(())
        for i, b in enumerate(seq):
            if i + 1 < len(seq):
                xf_n = emit_xload(seq[i + 1])
                nxt = stage1(seq[i + 1], xf_n, states[i + 1])
            else:
                nxt = iter(())
            emit_attention(b, states[i], chain(prev_proj, nxt))
            prev_proj = proj_gen(b, states[i])
        for _ in prev_proj:
            pass

    nc.compile()
    return nc


def _get_nc():
    if "nc" not in _NC_CACHE:
        _NC_CACHE["nc"] = _build_nc()
    return _NC_CACHE["nc"]


def kernel(x, qkv_w, qkv_b, proj_w, proj_b, t_h=8, t_w=8, s_h=16, s_w=16):
    from concourse.bass_utils import run_bass_kernel_spmd

    x = np.ascontiguousarray(np.asarray(x, dtype=np.float32))
    qkv_w = np.ascontiguousarray(np.asarray(qkv_w, dtype=np.float32))
    qkv_b = np.ascontiguousarray(np.asarray(qkv_b, dtype=np.float32))
    proj_w = np.ascontiguousarray(np.asarray(proj_w, dtype=np.float32))
    proj_b = np.ascontiguousarray(np.asarray(proj_b, dtype=np.float32))

    nc = _get_nc()
    in_maps = [
        {
            "x": x[i * BL:(i + 1) * BL],
            "qkv_w": qkv_w,
            "qkv_b": qkv_b,
            "proj_w": proj_w,
            "proj_b": proj_b,
        }
        for i in range(NCORES)
    ]
    res = run_bass_kernel_spmd(nc, in_maps, core_ids=list(range(NCORES)))
    out = np.concatenate([res.results[i]["out"] for i in range(NCORES)], axis=0)
    return out.astype(np.float32)
